# revision 2
# baseline (speedup 1.0000x reference)
"""Single-head causal attention (B=8, T=4096, EMB=1024, HEAD=64) on 8 trn2 cores.

Strategy: data-parallel over batch, one batch element per NeuronCore.

Per core (all matmuls in bf16, fp32 PSUM accumulation):
  1. QKV projection from host-pretransposed xT [1024, 4096]:
       KQ^T [128, 4096]  (rows 0:64 = K^T, 64:128 = Q^T), via W[:, 0:128] stationary
       V    [4096, 64]   natural layout, via xT-chunk stationary x Wv moving
  2. Q^T copied to partitions 0:63 (sbuf->sbuf DMA) so scores matmuls can
     contract over d=64 on partitions 0:63.
  3. Scores S^T[s, t] = K @ Q^T per (s-chunk 128, t-tile 512), PSUM fp32.
     No max-subtraction: scores ~ N(0, 0.41), exp is safe in fp32.
  4. exp via ScalarE directly from PSUM (scale=1/8 folded in), bf16 out.
     Causal: only tiles with t >= s computed; diagonal 128x128 block masked
     by a 0/1 multiply after exp.
  5. PV: P^T tile [128s, 128t] stationary, V-with-ones-column [128, 65] moving
     -> accumulates [O_unnorm | rowsum] in natural [t, 65] layout.
  6. out = O_unnorm * reciprocal(rowsum) (per-partition scalar), DMA out fp32.
"""

from contextlib import ExitStack

import numpy as np
import ml_dtypes

B, T, EMB, HEAD = 8, 4096, 1024, 64
KCH = EMB // 128          # 8 contraction chunks
NTT = T // 512            # 8 t-tiles of 512
NTS = T // 128            # 32 t-subtiles / s-chunks of 128
BF16 = ml_dtypes.bfloat16

_CACHE = {}


def _build_program():
    import concourse.bacc as bacc
    import concourse.tile as tile
    from concourse import mybir
    from concourse.masks import make_identity

    fp32 = mybir.dt.float32
    bf16 = mybir.dt.bfloat16
    EXP = mybir.ActivationFunctionType.Exp

    nc = bacc.Bacc("TRN2", target_bir_lowering=False, debug=False)
    xt_ap = nc.dram_tensor("xt", [EMB, T], bf16, kind="ExternalInput").ap()
    w_ap = nc.dram_tensor("w", [EMB, 192], bf16, kind="ExternalInput").ap()
    mask_ap = nc.dram_tensor("mask", [128, 128], bf16, kind="ExternalInput").ap()
    o_ap = nc.dram_tensor("o", [T, HEAD], fp32, kind="ExternalOutput").ap()

    with tile.TileContext(nc) as tc:
        with (
            tc.tile_pool(name="consts", bufs=1) as consts,
            tc.tile_pool(name="outs", bufs=4) as outs,
        ):
            # ---------- constants ----------
            w_sb = consts.tile([128, KCH, 192], bf16, tag="w")
            for k in range(KCH):
                nc.sync.dma_start(out=w_sb[:, k, :], in_=w_ap[k * 128:(k + 1) * 128, :])
            mask_sb = consts.tile([128, 128], bf16, tag="mask")
            nc.sync.dma_start(out=mask_sb, in_=mask_ap)
            ident_sb = consts.tile([128, 128], fp32, tag="ident")
            make_identity(nc, ident_sb)
            # V with ones column: [128, 65] per s-chunk; col 64 preset to 1.0
            vt_sb = consts.tile([128, NTS * 65], bf16, tag="vt")
            nc.gpsimd.memset(vt_sb, 1.0)

            kq_sb = consts.tile([128, T], bf16, tag="kq")
            qk_sb = consts.tile([128, T], bf16, tag="qk")

            # ---------- phase 1: load x, project (k-outer: PE starts as
            # each x chunk lands; KQ in two half-T passes + V share 8 banks)
            with (
                tc.tile_pool(name="xp", bufs=1) as xp,
                tc.tile_pool(name="ps_kq", bufs=1, space="PSUM") as ps_kq,
                tc.tile_pool(name="ps_v", bufs=1, space="PSUM") as ps_v,
            ):
                xt_sb = xp.tile([128, KCH, T], bf16, tag="xt")
                for k in range(KCH):
                    nc.sync.dma_start(
                        out=xt_sb[:, k, :], in_=xt_ap[k * 128:(k + 1) * 128, :]
                    )

                pkq = []
                for j in range(4):
                    pkq_j = ps_kq.tile([128, 512], fp32, tag=f"kq{j}")
                    pkq.append(pkq_j)
                pv = []
                for g in range(4):
                    pv_g = ps_v.tile([128, 512], fp32, tag=f"v{g}")
                    pv.append(pv_g)

                # pass 1 over k: KQ t-tiles 0..3 and all V accumulators
                for k in range(KCH):
                    for j in range(4):
                        nc.tensor.matmul(
                            pkq[j],
                            w_sb[:, k, 0:128],
                            xt_sb[:, k, j * 512:(j + 1) * 512],
                            start=(k == 0),
                            stop=(k == KCH - 1),
                            skip_group_check=True,
                        )
                    for i in range(NTS):
                        # start=True clears the WHOLE bank's has_written bits,
                        # so only the first accumulator in each bank may issue
                        # it; the rest overwrite-on-first-write via per-element
                        # has_written.
                        nc.tensor.matmul(
                            pv[i // 8][:, (i % 8) * 64:(i % 8 + 1) * 64],
                            xt_sb[:, k, i * 128:(i + 1) * 128],
                            w_sb[:, k, 128:192],
                            start=(k == 0 and i % 8 == 0),
                            stop=(k == KCH - 1),
                            skip_group_check=True,
                        )
                for j in range(4):
                    nc.vector.tensor_copy(kq_sb[:, j * 512:(j + 1) * 512], pkq[j])
                for i in range(NTS):
                    nc.vector.tensor_copy(
                        vt_sb[:, i * 65:i * 65 + 64],
                        pv[i // 8][:, (i % 8) * 64:(i % 8 + 1) * 64],
                    )
                # pass 2 over k (x fully resident): KQ t-tiles 4..7
                pkq2 = []
                for j in range(4):
                    pkq2_j = ps_kq.tile([128, 512], fp32, tag=f"kq{j}")
                    pkq2.append(pkq2_j)
                for k in range(KCH):
                    for j in range(4):
                        nc.tensor.matmul(
                            pkq2[j],
                            w_sb[:, k, 0:128],
                            xt_sb[:, k, (j + 4) * 512:(j + 5) * 512],
                            start=(k == 0),
                            stop=(k == KCH - 1),
                            skip_group_check=True,
                        )
                for j in range(4):
                    nc.vector.tensor_copy(
                        kq_sb[:, (j + 4) * 512:(j + 5) * 512], pkq2[j]
                    )
                # Q^T to low partitions for scores moving operand
                nc.sync.dma_start(out=qk_sb[0:64, :], in_=kq_sb[64:128, :])

            # ---------- phase 2: attention ----------
            phase2 = ExitStack()
            ptp = phase2.enter_context(tc.tile_pool(name="pt", bufs=1))
            ps_s = phase2.enter_context(tc.tile_pool(name="ps_s", bufs=2, space="PSUM"))
            ps_o = phase2.enter_context(tc.tile_pool(name="ps_o", bufs=1, space="PSUM"))
            pt = []
            for a in range(NTS):
                pt_a = ptp.tile([128, T - 128 * a], bf16, tag=f"pt{a}")
                pt.append(pt_a)

            def score_groups(a):
                """[(jstart, gsize), ...] groups of <=3 t-tiles for s-chunk a."""
                j0 = a // 4
                groups = []
                j = j0
                while j < NTT:
                    g = min(3, NTT - j)
                    groups.append((j, g))
                    j += g
                return groups

            def emit_scores(a):
                tiles = []
                for (jstart, g) in score_groups(a):
                    psg = ps_s.tile([128, 512 * g], fp32, tag="sg")
                    for idx in range(g):
                        j = jstart + idx
                        nc.tensor.matmul(
                            psg[:, idx * 512:(idx + 1) * 512],
                            kq_sb[0:64, a * 128:(a + 1) * 128],
                            qk_sb[0:64, j * 512:(j + 1) * 512],
                            start=True,
                            stop=True,
                        )
                    tiles.append((jstart, g, psg))
                return tiles

            def emit_exp(a, tiles):
                for (jstart, g, psg) in tiles:
                    skip = max(0, 128 * a - 512 * jstart)
                    out_lo = 512 * jstart + skip - 128 * a
                    out_hi = 512 * (jstart + g) - 128 * a
                    nc.scalar.activation(
                        pt[a][:, out_lo:out_hi],
                        psg[:, skip:512 * g],
                        EXP,
                        scale=0.125,
                    )
                # mask the diagonal 128x128 block (zero where s > t)
                nc.vector.tensor_mul(pt[a][:, 0:128], pt[a][:, 0:128], mask_sb)

            def emit_pv(i):
                po = ps_o.tile([128, 65], fp32, tag="o")
                for aa in range(i + 1):
                    nc.tensor.matmul(
                        po,
                        pt[aa][:, 128 * (i - aa):128 * (i - aa) + 128],
                        vt_sb[:, aa * 65:(aa + 1) * 65],
                        start=(aa == 0),
                        stop=(aa == i),
                    )
                dr = outs.tile([128, 1], fp32, tag="dr")
                nc.vector.reciprocal(dr, po[:, 64:65])
                o_sb = outs.tile([128, 64], fp32, tag="o_sb")
                nc.vector.tensor_scalar_mul(o_sb, po[:, 0:64], dr)
                nc.sync.dma_start(out=o_ap[i * 128:(i + 1) * 128, :], in_=o_sb)

            # software-pipelined: while ACT(a) drains, PE runs S(a+1); PV for
            # t-tile j fires once its last needed chunk (4j+3) is exp'd.
            tiles = emit_scores(0)
            for a in range(NTS):
                emit_exp(a, tiles)
                if a + 1 < NTS:
                    tiles = emit_scores(a + 1)
                if a >= 1:
                    emit_pv(a - 1)
            emit_pv(NTS - 1)
            phase2.close()

    nc.compile()
    return nc


def _get_nc():
    if "nc" not in _CACHE:
        _CACHE["nc"] = _build_program()
    return _CACHE["nc"]


def _in_maps(x, W):
    x = np.asarray(x, dtype=np.float32)
    W = np.asarray(W, dtype=np.float32)
    assert x.shape == (B, T, EMB) and W.shape == (EMB, 3 * HEAD)

    xt = np.ascontiguousarray(x.transpose(0, 2, 1)).astype(BF16)  # [B, EMB, T]
    w16 = W.astype(BF16)
    mask = np.triu(np.ones((128, 128), np.float32)).astype(BF16)
    return [{"xt": xt[b], "w": w16, "mask": mask} for b in range(B)]


def kernel(x, W):
    from concourse.bass_utils import run_bass_kernel_spmd

    nc = _get_nc()
    res = run_bass_kernel_spmd(nc, _in_maps(x, W), list(range(B)))
    return np.stack([res.results[b]["o"] for b in range(B)]).astype(np.float32)



# revision 7
# speedup vs baseline: 1.0333x; 1.0333x over previous
"""Single-head causal attention (B=8, T=4096, EMB=1024, HEAD=64) on 8 trn2 cores.

Strategy: data-parallel over batch, one batch element per NeuronCore.

Per core (all matmuls bf16 with fp32 PSUM):
  Phase 1 -- quarter-streamed QKV projection (DMA-paced):
    x^T arrives in 4 t-quarters of [1024, 1024]; per quarter the PE computes
    KQ^T [128, 1024] (k-outer accumulation, rows 0:64 = K^T, 64:128 = Q^T)
    and V [1024, 64] into PSUM; VectorE evacuates to SBUF as bf16; DMA
    cross-copies swap the partition halves (kqB = [Q^T; K^T]) so scores can
    run 2x row-tiled.
  Phase 2 -- attention, PE in 64-row tile mode throughout (tile_position),
  staged in two halves so exp starts as soon as half the sequence is
  projected:
    Stage 1 (after t<2048 projected): scores for chunks a<16, tiles j<4.
    Stage 2: everything else + PV + normalize + output.
    Per chunk pair (a, a+1): scores S^T = K_a @ Q^T on PE tile (0,0) and
    K_{a+1} @ Q^T on tile (64,0) concurrently; VectorE casts PSUM->pt bf16;
    ScalarE does ONE big in-place exp(0.125*s) per (chunk, stage); GpSimd
    masks the diagonal 128x128 block; PV is split-K over the two PE tiles
    (s_lo/s_hi) with a VectorE add combining the halves; the ones-column in
    V yields the softmax rowsum, normalized via reciprocal + scalar-mul.
  Engine budget: ScalarE (exp, ~8.7M elems) is the pacing engine; VectorE
  casts/normalizes; GpSimd does masks + aux DMA issue; PE runs ~50% duty.
"""

from contextlib import ExitStack

import numpy as np
import ml_dtypes

B, T, EMB, HEAD = 8, 4096, 1024, 64
KCH = EMB // 128          # 8 contraction chunks
NQ = 4                    # x streamed in 4 t-quarters
QW = T // NQ              # 1024
NTT = T // 512            # 8 t-tiles of 512
NTS = T // 128            # 32 s-chunks of 128
BF16 = ml_dtypes.bfloat16

_CACHE = {}


def _build_program():
    import concourse.bacc as bacc
    import concourse.tile as tile
    from concourse import mybir

    fp32 = mybir.dt.float32
    bf16 = mybir.dt.bfloat16
    EXP = mybir.ActivationFunctionType.Exp

    nc = bacc.Bacc("TRN2", target_bir_lowering=False, debug=False)
    xt_ap = nc.dram_tensor("xt", [EMB, T], bf16, kind="ExternalInput").ap()
    w_ap = nc.dram_tensor("w", [EMB, 192], bf16, kind="ExternalInput").ap()
    mask_ap = nc.dram_tensor("mask", [128, 128], bf16, kind="ExternalInput").ap()
    o_ap = nc.dram_tensor("o", [T, HEAD], fp32, kind="ExternalOutput").ap()

    with tile.TileContext(nc) as tc:
        with (
            tc.tile_pool(name="consts", bufs=1) as consts,
            tc.tile_pool(name="outs", bufs=4) as outs,
        ):
            # ---------- constants ----------
            w_sb = consts.tile([128, KCH, 192], bf16, tag="w")
            for k in range(KCH):
                nc.sync.dma_start(out=w_sb[:, k, :], in_=w_ap[k * 128:(k + 1) * 128, :])
            mask_sb = consts.tile([128, 128], bf16, tag="mask")
            nc.sync.dma_start(out=mask_sb, in_=mask_ap)
            # V with ones column: [128, 32, 65]; col 64 stays 1.0 (rowsum)
            vt_sb = consts.tile([128, NTS, 65], bf16, tag="vt")
            nc.gpsimd.memset(vt_sb, 1.0)

            # kqA: rows 0:64 = K^T, 64:128 = Q^T (natural PE layout)
            # kqB: partition-halves swapped (for 2x row-tiled scores)
            kqA = consts.tile([128, T], bf16, tag="kqA")
            kqB = consts.tile([128, T], bf16, tag="kqB")

            # exp'd score rows: pt[a] covers t in [128a, T)
            ptp = ExitStack()
            ptpool = ptp.enter_context(tc.tile_pool(name="pt", bufs=1))
            pt = []
            for a in range(NTS):
                pt_a = ptpool.tile([128, T - 128 * a], bf16, tag=f"pt{a}")
                pt.append(pt_a)

            # ---------- phase 1 pools ----------
            phase1 = ExitStack()
            xp = phase1.enter_context(tc.tile_pool(name="xp", bufs=2))
            ps_kq = phase1.enter_context(
                tc.tile_pool(name="ps_kq", bufs=1, space="PSUM"))
            ps_v = phase1.enter_context(
                tc.tile_pool(name="ps_v", bufs=1, space="PSUM"))
            s1stack = ExitStack()
            ps_s1 = s1stack.enter_context(
                tc.tile_pool(name="ps_s1", bufs=2, space="PSUM"))

            def emit_quarter_dma(q):
                xq = xp.tile([128, KCH, QW], bf16, tag="x")
                for k in range(KCH):
                    eng = nc.sync if k % 2 == 0 else nc.gpsimd
                    eng.dma_start(
                        out=xq[:, k, :],
                        in_=xt_ap[k * 128:(k + 1) * 128, q * QW:(q + 1) * QW],
                    )
                return xq

            def emit_quarter_mm(q, xq, k):
                """PE work for x chunk (k, quarter q)."""
                pkq, pv = quarter_psum[q]
                for j2 in range(2):
                    nc.tensor.matmul(
                        pkq[:, j2 * 512:(j2 + 1) * 512],
                        w_sb[:, k, 0:128],
                        xq[:, k, j2 * 512:(j2 + 1) * 512],
                        start=(k == 0),
                        stop=(k == KCH - 1),
                        skip_group_check=True,
                    )
                for i in range(8):
                    nc.tensor.matmul(
                        pv[:, i, :],
                        xq[:, k, i * 128:(i + 1) * 128],
                        w_sb[:, k, 128:192],
                        start=(k == 0 and i == 0),
                        stop=(k == KCH - 1),
                        skip_group_check=True,
                    )

            def emit_quarter_evac(q):
                pkq, pv = quarter_psum[q]
                qsl = slice(q * QW, (q + 1) * QW)
                nc.vector.tensor_copy(kqA[:, qsl], pkq)
                nc.vector.tensor_copy(vt_sb[:, 8 * q:8 * q + 8, 0:64], pv)
                # partition-half swap for row tiling (gpsimd queue, off PE path)
                nc.gpsimd.dma_start(out=kqB[64:128, qsl], in_=kqA[0:64, qsl])
                nc.gpsimd.dma_start(out=kqB[0:64, qsl], in_=kqA[64:128, qsl])

            # ---------- phase 2 emitters ----------
            def emit_score_tile(pool, a0, a1, j):
                """One 512-wide t-tile for chunks a0 (PE tile 0,0) and a1
                (PE tile 64,0), into a 2-bank PSUM tile."""
                ps = pool.tile([128, 2, 512], fp32, tag="sg")
                nc.tensor.matmul(
                    ps[:, 0, :],
                    kqA[0:64, a0 * 128:(a0 + 1) * 128],
                    kqB[0:64, j * 512:(j + 1) * 512],
                    start=True, stop=True,
                    tile_position=(0, 0),
                )
                nc.tensor.matmul(
                    ps[:, 1, :],
                    kqB[64:128, a1 * 128:(a1 + 1) * 128],
                    kqA[64:128, j * 512:(j + 1) * 512],
                    start=True, stop=True,
                    tile_position=(64, 0),
                )
                # cast PSUM fp32 -> pt bf16 (raw scores), clipped at diagonal
                for idx, a in ((0, a0), (1, a1)):
                    skip = max(0, 128 * a - 512 * j)
                    if skip >= 512:
                        continue
                    nc.vector.tensor_copy(
                        pt[a][:, 512 * j + skip - 128 * a:512 * (j + 1) - 128 * a],
                        ps[:, idx, skip:512],
                    )

            def emit_exp(a, stage):
                w1 = max(0, 2048 - 128 * a)
                lo, hi = (0, w1) if stage == 1 else (w1, T - 128 * a)
                if hi <= lo:
                    return
                sl = pt[a][:, lo:hi]
                nc.scalar.activation(sl, sl, EXP, scale=0.125)
                if lo == 0:
                    # zero below-diagonal of the 128x128 diagonal block
                    nc.gpsimd.tensor_mul(pt[a][:, 0:128], pt[a][:, 0:128], mask_sb)

            def emit_pair_scores(pool, p, stage):
                a0, a1 = 2 * p, 2 * p + 1
                j0 = a0 // 4
                jlo, jhi = (j0, 4) if stage == 1 else (max(4, j0), NTT)
                for j in range(jlo, jhi):
                    emit_score_tile(pool, a0, a1, j)
                emit_exp(a0, stage)
                emit_exp(a1, stage)

            def emit_pv(i):
                po_x = ps_o.tile([128, 512], fp32, tag="ox")
                po_y = ps_o.tile([128, 512], fp32, tag="oy")
                for aa in range(i + 1):
                    tsl = slice(128 * (i - aa), 128 * (i - aa) + 128)
                    nc.tensor.matmul(
                        po_x[:, 0:65],
                        pt[aa][0:64, tsl],
                        vt_sb[0:64, aa, :],
                        start=(aa == 0), stop=(aa == i),
                        tile_position=(0, 0),
                        skip_group_check=True,
                    )
                    nc.tensor.matmul(
                        po_y[:, 0:65],
                        pt[aa][64:128, tsl],
                        vt_sb[64:128, aa, :],
                        start=(aa == 0), stop=(aa == i),
                        tile_position=(64, 0),
                        skip_group_check=True,
                    )
                o65 = outs.tile([128, 65], fp32, tag="o65")
                nc.vector.tensor_copy(o65, po_y[:, 0:65])
                nc.vector.tensor_add(o65, o65, po_x[:, 0:65])
                dr = outs.tile([128, 1], fp32, tag="dr")
                nc.vector.reciprocal(dr, o65[:, 64:65])
                o_sb = outs.tile([128, 64], fp32, tag="o_sb")
                nc.vector.tensor_scalar_mul(o_sb, o65[:, 0:64], dr)
                nc.gpsimd.dma_start(out=o_ap[i * 128:(i + 1) * 128, :], in_=o_sb)

            # ---------- emission schedule ----------
            quarter_psum = {}

            def open_quarter(q):
                pkq = ps_kq.tile([128, QW], fp32, tag="kq")
                pv = ps_v.tile([128, 8, 64], fp32, tag="v")
                quarter_psum[q] = (pkq, pv)

            # quarters 0,1: pure load+project
            xqs = {}
            xqs[0] = emit_quarter_dma(0)
            xqs[1] = emit_quarter_dma(1)
            open_quarter(0)
            for k in range(KCH):
                emit_quarter_mm(0, xqs[0], k)
            emit_quarter_evac(0)
            open_quarter(1)
            for k in range(KCH):
                emit_quarter_mm(1, xqs[1], k)
            emit_quarter_evac(1)

            # quarters 2,3 interleaved with stage-1 score pairs (which only
            # need kq quarters 0-1); pairs fill PE gaps while DMA streams.
            xqs[2] = emit_quarter_dma(2)
            open_quarter(2)
            for k in range(KCH):
                emit_quarter_mm(2, xqs[2], k)
                if k in (2, 5):
                    emit_pair_scores(ps_s1, {2: 0, 5: 1}[k], stage=1)
            emit_quarter_evac(2)
            xqs[3] = emit_quarter_dma(3)
            open_quarter(3)
            for k in range(KCH):
                emit_quarter_mm(3, xqs[3], k)
                if k in (1, 3, 5, 7):
                    emit_pair_scores(ps_s1, {1: 2, 3: 3, 5: 4, 7: 5}[k], stage=1)
            emit_quarter_evac(3)
            emit_pair_scores(ps_s1, 6, stage=1)
            emit_pair_scores(ps_s1, 7, stage=1)

            s1stack.close()
            phase1.close()

            # stage 2: remaining scores + PV, all PSUM now free
            phase2 = ExitStack()
            ps_s2 = phase2.enter_context(
                tc.tile_pool(name="ps_s2", bufs=2, space="PSUM"))
            ps_o = phase2.enter_context(
                tc.tile_pool(name="ps_o", bufs=2, space="PSUM"))
            for p in range(16):
                emit_pair_scores(ps_s2, p, stage=2)
                if p >= 1:
                    emit_pv(2 * (p - 1))
                    emit_pv(2 * (p - 1) + 1)
            emit_pv(30)
            emit_pv(31)
            phase2.close()
            ptp.close()

    nc.compile()
    return nc


def _get_nc():
    if "nc" not in _CACHE:
        _CACHE["nc"] = _build_program()
    return _CACHE["nc"]


def _in_maps(x, W):
    x = np.asarray(x, dtype=np.float32)
    W = np.asarray(W, dtype=np.float32)
    assert x.shape == (B, T, EMB) and W.shape == (EMB, 3 * HEAD)

    xt = np.ascontiguousarray(x.transpose(0, 2, 1)).astype(BF16)  # [B, EMB, T]
    w16 = W.astype(BF16)
    mask = np.triu(np.ones((128, 128), np.float32)).astype(BF16)
    return [{"xt": xt[b], "w": w16, "mask": mask} for b in range(B)]


def kernel(x, W):
    from concourse.bass_utils import run_bass_kernel_spmd

    nc = _get_nc()
    res = run_bass_kernel_spmd(nc, _in_maps(x, W), list(range(B)))
    return np.stack([res.results[b]["o"] for b in range(B)]).astype(np.float32)


# revision 14
# speedup vs baseline: 1.0699x; 1.0354x over previous
"""Single-head causal attention (B=8, T=4096, EMB=1024, HEAD=64) on 8 trn2 cores.

Strategy: data-parallel over batch, one batch element per NeuronCore.

Per core (all matmuls bf16 with fp32 PSUM):
  Phase 1 -- quarter-streamed QKV projection (DMA-paced):
    x^T arrives in 4 t-quarters of [1024, 1024]; per quarter the PE computes
    KQ^T [128, 1024] (k-outer accumulation, rows 0:64 = K^T, 64:128 = Q^T)
    and V [1024, 64] into PSUM; VectorE evacuates to SBUF as bf16; DMA
    cross-copies swap the partition halves (kqB = [Q^T; K^T]) so scores can
    run 2x row-tiled.
  Phase 2 -- attention, PE in 64-row tile mode throughout (tile_position),
  staged in two halves so exp starts as soon as half the sequence is
  projected:
    Stage 1 (after t<2048 projected): scores for chunks a<16, tiles j<4.
    Stage 2: everything else + PV + normalize + output.
    Per chunk pair (a, a+1): scores S^T = K_a @ Q^T on PE tile (0,0) and
    K_{a+1} @ Q^T on tile (64,0) concurrently into multi-bank PSUM groups;
    ScalarE exps each group straight from PSUM into pt[a] bf16 (scale=1/8
    folded in); GpSimd masks the diagonal 128x128 block; PV is split-K over
    the two PE tiles (s_lo/s_hi) with a VectorE add combining the halves;
    the ones-column in V yields the softmax rowsum (reciprocal + mul).
  Engine budget: ScalarE (exp, ~8.7M elems at 1 elem/cycle/lane @1.2GHz) is
  the pacing engine; VectorE evacuates/normalizes; GpSimd masks + aux DMA.
"""

from contextlib import ExitStack

import numpy as np
import ml_dtypes

B, T, EMB, HEAD = 8, 4096, 1024, 64
KCH = EMB // 128          # 8 contraction chunks
NQ = 4                    # x streamed in 4 t-quarters
QW = T // NQ              # 1024
NTT = T // 512            # 8 t-tiles of 512
NTS = T // 128            # 32 s-chunks of 128
BF16 = ml_dtypes.bfloat16

_CACHE = {}


def _build_program():
    import concourse.bacc as bacc
    import concourse.tile as tile
    from concourse import mybir

    fp32 = mybir.dt.float32
    bf16 = mybir.dt.bfloat16
    EXP = mybir.ActivationFunctionType.Exp

    nc = bacc.Bacc("TRN2", target_bir_lowering=False, debug=False)
    # x^T pre-split into contiguous t-quarters on host: [NQ, EMB, QW]
    xt_ap = nc.dram_tensor("xt", [NQ, EMB, QW], bf16, kind="ExternalInput").ap()
    w_ap = nc.dram_tensor("w", [EMB, 192], bf16, kind="ExternalInput").ap()
    mask_ap = nc.dram_tensor("mask", [128, 128], bf16, kind="ExternalInput").ap()
    o_ap = nc.dram_tensor("o", [T, HEAD], fp32, kind="ExternalOutput").ap()

    with tile.TileContext(nc) as tc:
        with (
            tc.tile_pool(name="consts", bufs=1) as consts,
            tc.tile_pool(name="outs", bufs=4) as outs,
        ):
            # ---------- constants ----------
            w_sb = consts.tile([128, KCH, 192], bf16, tag="w")
            for k in range(KCH):
                nc.sync.dma_start(out=w_sb[:, k, :], in_=w_ap[k * 128:(k + 1) * 128, :])
            mask_sb = consts.tile([128, 128], bf16, tag="mask")
            nc.sync.dma_start(out=mask_sb, in_=mask_ap)
            # V with ones column: [128, 32, 65]; col 64 stays 1.0 (rowsum)
            vt_sb = consts.tile([128, NTS, 65], bf16, tag="vt")
            nc.gpsimd.memset(vt_sb, 1.0)

            # kqA: rows 0:64 = K^T, 64:128 = Q^T (natural PE layout)
            # kqB: partition-halves swapped (for 2x row-tiled scores)
            kqA = consts.tile([128, T], bf16, tag="kqA")
            kqB = consts.tile([128, T], bf16, tag="kqB")

            # exp'd score rows: pt[a] covers t in [128a, T)
            ptp = ExitStack()
            ptpool = ptp.enter_context(tc.tile_pool(name="pt", bufs=1))
            pt = []
            for a in range(NTS):
                pt_a = ptpool.tile([128, T - 128 * a], bf16, tag=f"pt{a}")
                pt.append(pt_a)

            # ---------- phase 1 pools ----------
            phase1 = ExitStack()
            xp = phase1.enter_context(tc.tile_pool(name="xp", bufs=2))
            ps_kq = phase1.enter_context(
                tc.tile_pool(name="ps_kq", bufs=1, space="PSUM"))
            ps_v = phase1.enter_context(
                tc.tile_pool(name="ps_v", bufs=1, space="PSUM"))
            s1stack = ExitStack()
            ps_s1 = s1stack.enter_context(
                tc.tile_pool(name="ps_s1", bufs=2, space="PSUM"))

            def emit_quarter_dma(q):
                xq = xp.tile([128, KCH, QW], bf16, tag="x")
                for k in range(KCH):
                    eng = nc.sync if k % 2 == 0 else nc.gpsimd
                    eng.dma_start(
                        out=xq[:, k, :],
                        in_=xt_ap[q, k * 128:(k + 1) * 128, :],
                    )
                return xq

            def emit_quarter_mm(q, xq, k):
                """PE work for x chunk (k, quarter q)."""
                pkq, pv = quarter_psum[q]
                for j2 in range(2):
                    nc.tensor.matmul(
                        pkq[:, j2 * 512:(j2 + 1) * 512],
                        w_sb[:, k, 0:128],
                        xq[:, k, j2 * 512:(j2 + 1) * 512],
                        start=(k == 0),
                        stop=(k == KCH - 1),
                        skip_group_check=True,
                    )
                for i in range(8):
                    nc.tensor.matmul(
                        pv[:, i, :],
                        xq[:, k, i * 128:(i + 1) * 128],
                        w_sb[:, k, 128:192],
                        start=(k == 0 and i == 0),
                        stop=(k == KCH - 1),
                        skip_group_check=True,
                    )

            def emit_quarter_evac(q):
                pkq, pv = quarter_psum[q]
                qsl = slice(q * QW, (q + 1) * QW)
                nc.vector.tensor_copy(kqA[:, qsl], pkq)
                nc.vector.tensor_copy(vt_sb[:, 8 * q:8 * q + 8, 0:64], pv)
                # partition-half swap for row tiling (gpsimd queue, off PE path)
                nc.gpsimd.dma_start(out=kqB[64:128, qsl], in_=kqA[0:64, qsl])
                nc.gpsimd.dma_start(out=kqB[0:64, qsl], in_=kqA[64:128, qsl])

            # ---------- phase 2 emitters ----------
            def emit_score_group(pool, a, jstart, g, pe_tile):
                """Scores for chunk a over t-tiles [jstart, jstart+g), then
                exp straight from PSUM into pt[a]. pe_tile 0 -> PE rows 0:64
                (operands in kqA/kqB low halves), 1 -> rows 64:128."""
                ps = pool.tile([128, g * 512], fp32, tag="sg")
                if pe_tile == 0:
                    kt, qt, psl = kqA, kqB, slice(0, 64)
                else:
                    kt, qt, psl = kqB, kqA, slice(64, 128)
                for idx in range(g):
                    j = jstart + idx
                    nc.tensor.matmul(
                        ps[:, idx * 512:(idx + 1) * 512],
                        kt[psl, a * 128:(a + 1) * 128],
                        qt[psl, j * 512:(j + 1) * 512],
                        start=True, stop=True,
                        tile_position=(64 * pe_tile, 0),
                    )
                skip = max(0, 128 * a - 512 * jstart)
                nc.scalar.activation(
                    pt[a][:, 512 * jstart + skip - 128 * a:512 * (jstart + g) - 128 * a],
                    ps[:, skip:g * 512],
                    EXP,
                    scale=0.125,
                )
                if skip > 0 or 512 * jstart == 128 * a:
                    # group contains the diagonal: zero below-diagonal block
                    nc.gpsimd.tensor_mul(pt[a][:, 0:128], pt[a][:, 0:128], mask_sb)

            def emit_pair_scores(pool, p, stage, gmax):
                a0, a1 = 2 * p, 2 * p + 1
                j0 = a0 // 4
                jlo, jhi = (j0, 4) if stage == 1 else (max(4, j0), NTT)
                groups = []
                j = jlo
                while j < jhi:
                    g = min(gmax, jhi - j)
                    groups.append((j, g))
                    j += g
                for (jstart, g) in groups:
                    emit_score_group(pool, a0, jstart, g, pe_tile=0)
                    emit_score_group(pool, a1, jstart, g, pe_tile=1)

            def emit_pv(i):
                po_x = ps_o.tile([128, 512], fp32, tag="ox")
                po_y = ps_o.tile([128, 512], fp32, tag="oy")
                for aa in range(i + 1):
                    tsl = slice(128 * (i - aa), 128 * (i - aa) + 128)
                    nc.tensor.matmul(
                        po_x[:, 0:65],
                        pt[aa][0:64, tsl],
                        vt_sb[0:64, aa, :],
                        start=(aa == 0), stop=(aa == i),
                        tile_position=(0, 0),
                        skip_group_check=True,
                    )
                    nc.tensor.matmul(
                        po_y[:, 0:65],
                        pt[aa][64:128, tsl],
                        vt_sb[64:128, aa, :],
                        start=(aa == 0), stop=(aa == i),
                        tile_position=(64, 0),
                        skip_group_check=True,
                    )
                o65 = outs.tile([128, 65], fp32, tag="o65")
                nc.vector.tensor_copy(o65, po_y[:, 0:65])
                nc.vector.tensor_add(o65, o65, po_x[:, 0:65])
                dr = outs.tile([128, 1], fp32, tag="dr")
                nc.vector.reciprocal(dr, o65[:, 64:65])
                o_sb = outs.tile([128, 64], fp32, tag="o_sb")
                nc.vector.tensor_scalar_mul(o_sb, o65[:, 0:64], dr)
                nc.sync.dma_start(out=o_ap[i * 128:(i + 1) * 128, :], in_=o_sb)

            # ---------- emission schedule ----------
            quarter_psum = {}

            def open_quarter(q):
                pkq = ps_kq.tile([128, QW], fp32, tag="kq")
                pv = ps_v.tile([128, 8, 64], fp32, tag="v")
                quarter_psum[q] = (pkq, pv)

            # quarters 0,1: pure load+project
            xqs = {}
            xqs[0] = emit_quarter_dma(0)
            xqs[1] = emit_quarter_dma(1)
            open_quarter(0)
            for k in range(KCH):
                emit_quarter_mm(0, xqs[0], k)
            emit_quarter_evac(0)
            open_quarter(1)
            for k in range(KCH):
                emit_quarter_mm(1, xqs[1], k)
            emit_quarter_evac(1)

            # quarters 2,3 interleaved with stage-1 score pairs (which only
            # need kq quarters 0-1); pairs fill PE gaps while DMA streams.
            xqs[2] = emit_quarter_dma(2)
            open_quarter(2)
            for k in range(KCH):
                emit_quarter_mm(2, xqs[2], k)
                if k in (2, 5):
                    emit_pair_scores(ps_s1, {2: 0, 5: 1}[k], stage=1, gmax=2)
            emit_quarter_evac(2)
            xqs[3] = emit_quarter_dma(3)
            open_quarter(3)
            for k in range(KCH):
                emit_quarter_mm(3, xqs[3], k)
                if k in (1, 3, 5, 7):
                    emit_pair_scores(ps_s1, {1: 2, 3: 3, 5: 4, 7: 5}[k], stage=1, gmax=2)
            emit_quarter_evac(3)
            emit_pair_scores(ps_s1, 6, stage=1, gmax=2)
            emit_pair_scores(ps_s1, 7, stage=1, gmax=2)

            s1stack.close()
            phase1.close()

            # stage 2: remaining scores + PV, all PSUM now free
            phase2 = ExitStack()
            ps_s2 = phase2.enter_context(
                tc.tile_pool(name="ps_s2", bufs=2, space="PSUM"))
            ps_o = phase2.enter_context(
                tc.tile_pool(name="ps_o", bufs=1, space="PSUM"))
            for p in range(16):
                emit_pair_scores(ps_s2, p, stage=2, gmax=3)
                if p >= 1:
                    emit_pv(2 * (p - 1))
                    emit_pv(2 * (p - 1) + 1)
            emit_pv(30)
            emit_pv(31)
            phase2.close()
            ptp.close()

    nc.compile()
    return nc


def _get_nc():
    if "nc" not in _CACHE:
        _CACHE["nc"] = _build_program()
    return _CACHE["nc"]


def _in_maps(x, W):
    x = np.asarray(x, dtype=np.float32)
    W = np.asarray(W, dtype=np.float32)
    assert x.shape == (B, T, EMB) and W.shape == (EMB, 3 * HEAD)

    xt = x.transpose(0, 2, 1)  # [B, EMB, T]
    # quarter-contiguous: [B, NQ, EMB, QW] so each DMA line is contiguous
    xtq = np.ascontiguousarray(
        xt.reshape(B, EMB, NQ, QW).transpose(0, 2, 1, 3)
    ).astype(BF16)
    w16 = W.astype(BF16)
    mask = np.triu(np.ones((128, 128), np.float32)).astype(BF16)
    return [{"xt": xtq[b], "w": w16, "mask": mask} for b in range(B)]


def kernel(x, W):
    from concourse.bass_utils import run_bass_kernel_spmd

    nc = _get_nc()
    res = run_bass_kernel_spmd(nc, _in_maps(x, W), list(range(B)))
    return np.stack([res.results[b]["o"] for b in range(B)]).astype(np.float32)


# revision 20
# speedup vs baseline: 1.2091x; 1.1301x over previous
"""Single-head causal attention (B=8, T=4096, EMB=1024, HEAD=64) on 8 trn2 cores.

Strategy: data-parallel over batch, one batch element per NeuronCore.

Per core (all matmuls bf16 with fp32 PSUM):
  Phase 1 -- quarter-streamed QKV projection (DMA-paced):
    x^T arrives in 4 t-quarters of [1024, 1024]; per quarter the PE computes
    KQ^T [128, 1024] (k-outer accumulation, rows 0:64 = K^T, 64:128 = Q^T)
    and V [1024, 64] into PSUM; VectorE evacuates to SBUF as bf16; DMA
    cross-copies swap the partition halves (kqB = [Q^T; K^T]) so scores can
    run 2x row-tiled.
  Phase 2 -- attention, PE in 64-row tile mode throughout (tile_position),
  staged in two halves so exp starts as soon as half the sequence is
  projected:
    Stage 1 (after t<2048 projected): scores for chunks a<16, tiles j<4.
    Stage 2: everything else + PV + normalize + output.
    Per chunk pair (a, a+1): scores S^T = K_a @ Q^T on PE tile (0,0) and
    K_{a+1} @ Q^T on tile (64,0) concurrently into multi-bank PSUM groups;
    ScalarE exps each group straight from PSUM into pt[a] bf16 (scale=1/8
    folded in); GpSimd masks the diagonal 128x128 block; PV is split-K over
    the two PE tiles (s_lo/s_hi) with a VectorE add combining the halves;
    the ones-column in V yields the softmax rowsum (reciprocal + mul).
  Engine budget: ScalarE (exp, ~8.7M elems at 1 elem/cycle/lane @1.2GHz) is
  the pacing engine; VectorE evacuates/normalizes; GpSimd masks + aux DMA.
"""

from contextlib import ExitStack

import numpy as np
import ml_dtypes

B, T, EMB, HEAD = 8, 4096, 1024, 64
KCH = EMB // 128          # 8 contraction chunks
NQ = 4                    # x streamed in 4 t-quarters
QW = T // NQ              # 1024
NTT = T // 512            # 8 t-tiles of 512
NTS = T // 128            # 32 s-chunks of 128
BF16 = ml_dtypes.bfloat16

_CACHE = {}


def _build_program():
    import concourse.bacc as bacc
    import concourse.tile as tile
    from concourse import mybir

    fp32 = mybir.dt.float32
    bf16 = mybir.dt.bfloat16
    EXP = mybir.ActivationFunctionType.Exp

    nc = bacc.Bacc("TRN2", target_bir_lowering=False, debug=False)
    # x^T pre-split into contiguous t-quarters on host: [NQ, EMB, QW]
    xt_ap = nc.dram_tensor("xt", [NQ, EMB, QW], bf16, kind="ExternalInput").ap()
    w_ap = nc.dram_tensor("w", [EMB, 192], bf16, kind="ExternalInput").ap()
    mask_ap = nc.dram_tensor("mask", [128, 128], bf16, kind="ExternalInput").ap()
    # O^T accumulator layout: row 64 = softmax rowsum; host divides+transposes
    o_ap = nc.dram_tensor("o", [HEAD + 1, T], fp32, kind="ExternalOutput").ap()

    with tile.TileContext(nc) as tc:
        with (
            tc.tile_pool(name="consts", bufs=1) as consts,
            tc.tile_pool(name="outs", bufs=4) as outs,
        ):
            # ---------- constants ----------
            w_sb = consts.tile([128, KCH, 192], bf16, tag="w")
            for k in range(KCH):
                nc.sync.dma_start(out=w_sb[:, k, :], in_=w_ap[k * 128:(k + 1) * 128, :])
            mask_sb = consts.tile([128, 128], bf16, tag="mask")
            nc.sync.dma_start(out=mask_sb, in_=mask_ap)
            # V with ones column: [128, 32, 65]; col 64 stays 1.0 (rowsum)
            vt_sb = consts.tile([128, NTS, 65], bf16, tag="vt")
            nc.gpsimd.memset(vt_sb, 1.0)

            # kqA: rows 0:64 = K^T, 64:128 = Q^T (natural PE layout)
            # kqB: partition-halves swapped (for 2x row-tiled scores)
            kqA = consts.tile([128, T], bf16, tag="kqA")
            kqB = consts.tile([128, T], bf16, tag="kqB")

            # exp'd score rows: pt[a] covers t in [128a, T)
            ptp = ExitStack()
            ptpool = ptp.enter_context(tc.tile_pool(name="pt", bufs=1))
            pt = []
            for a in range(NTS):
                pt_a = ptpool.tile([128, T - 128 * a], bf16, tag=f"pt{a}")
                pt.append(pt_a)

            # ---------- phase 1 pools ----------
            phase1 = ExitStack()
            xp = phase1.enter_context(tc.tile_pool(name="xp", bufs=2))
            ps_kq = phase1.enter_context(
                tc.tile_pool(name="ps_kq", bufs=1, space="PSUM"))
            ps_v = phase1.enter_context(
                tc.tile_pool(name="ps_v", bufs=1, space="PSUM"))
            s1stack = ExitStack()
            ps_s1 = s1stack.enter_context(
                tc.tile_pool(name="ps_s1", bufs=2, space="PSUM"))

            def emit_quarter_dma(q):
                # per-chunk tiles so chunk (q+2, k) can start as soon as
                # (q, k)'s matmuls have read it (no quarter-boundary drain)
                xq = {}
                for k in range(KCH):
                    xq_k = xp.tile([128, QW], bf16, tag=f"x{k}")
                    eng = nc.sync if k % 2 == 0 else nc.gpsimd
                    eng.dma_start(
                        out=xq_k,
                        in_=xt_ap[q, k * 128:(k + 1) * 128, :],
                    )
                    xq[k] = xq_k
                return xq

            def emit_quarter_mm(q, xq, k):
                """PE work for x chunk (k, quarter q)."""
                pkq, pv = quarter_psum[q]
                for j2 in range(2):
                    nc.tensor.matmul(
                        pkq[:, j2 * 512:(j2 + 1) * 512],
                        w_sb[:, k, 0:128],
                        xq[k][:, j2 * 512:(j2 + 1) * 512],
                        start=(k == 0),
                        stop=(k == KCH - 1),
                        skip_group_check=True,
                    )
                for i in range(8):
                    nc.tensor.matmul(
                        pv[:, i, :],
                        xq[k][:, i * 128:(i + 1) * 128],
                        w_sb[:, k, 128:192],
                        start=(k == 0 and i == 0),
                        stop=(k == KCH - 1),
                        skip_group_check=True,
                    )

            def emit_quarter_evac(q):
                pkq, pv = quarter_psum[q]
                qsl = slice(q * QW, (q + 1) * QW)
                nc.vector.tensor_copy(kqA[:, qsl], pkq)
                nc.vector.tensor_copy(vt_sb[:, 8 * q:8 * q + 8, 0:64], pv)
                # partition-half swap for row tiling (gpsimd queue, off PE path)
                nc.gpsimd.dma_start(out=kqB[64:128, qsl], in_=kqA[0:64, qsl])
                nc.gpsimd.dma_start(out=kqB[0:64, qsl], in_=kqA[64:128, qsl])

            # ---------- phase 2 emitters ----------
            def emit_score_group(pool, a, jstart, g, pe_tile):
                """Scores for chunk a over t-tiles [jstart, jstart+g), then
                exp straight from PSUM into pt[a]. pe_tile 0 -> PE rows 0:64
                (operands in kqA/kqB low halves), 1 -> rows 64:128."""
                ps = pool.tile([128, g * 512], fp32, tag="sg")
                if pe_tile == 0:
                    kt, qt, psl = kqA, kqB, slice(0, 64)
                else:
                    kt, qt, psl = kqB, kqA, slice(64, 128)
                for idx in range(g):
                    j = jstart + idx
                    nc.tensor.matmul(
                        ps[:, idx * 512:(idx + 1) * 512],
                        kt[psl, a * 128:(a + 1) * 128],
                        qt[psl, j * 512:(j + 1) * 512],
                        start=True, stop=True,
                        tile_position=(64 * pe_tile, 0),
                    )
                skip = max(0, 128 * a - 512 * jstart)
                nc.scalar.activation(
                    pt[a][:, 512 * jstart + skip - 128 * a:512 * (jstart + g) - 128 * a],
                    ps[:, skip:g * 512],
                    EXP,
                    scale=0.125,
                )
                if skip > 0 or 512 * jstart == 128 * a:
                    # group contains the diagonal: zero below-diagonal block
                    nc.gpsimd.tensor_mul(pt[a][:, 0:128], pt[a][:, 0:128], mask_sb)

            def emit_pair_scores(pool, p, stage, gmax):
                a0, a1 = 2 * p, 2 * p + 1
                j0 = a0 // 4
                jlo, jhi = (j0, 4) if stage == 1 else (max(4, j0), NTT)
                groups = []
                j = jlo
                while j < jhi:
                    g = min(gmax, jhi - j)
                    groups.append((j, g))
                    j += g
                for (jstart, g) in groups:
                    emit_score_group(pool, a0, jstart, g, pe_tile=0)
                    emit_score_group(pool, a1, jstart, g, pe_tile=1)

            def emit_pv(j):
                """O^T[:, 512j:512j+512] = sum_aa V_aa^T @ P_aa^T, split-K
                over the two PE row tiles (s_lo on (0,0), s_hi on (64,0)),
                vt stationary (65 cols, reused-shape) and pt moving (up to
                512 cols) so LDWEIGHTS hides behind the matmul stream."""
                po_x = ps_o.tile([128, 512], fp32, tag="ox")
                po_y = ps_o.tile([128, 512], fp32, tag="oy")
                last = 4 * j + 3
                for aa in range(last + 1):
                    skip = max(0, 128 * aa - 512 * j)
                    lo = 512 * j - 128 * aa + skip
                    for (po, psl, tp) in (
                        (po_x, slice(0, 64), (0, 0)),
                        (po_y, slice(64, 128), (64, 0)),
                    ):
                        nc.tensor.matmul(
                            po[0:65, skip:512],
                            vt_sb[psl, aa, :],
                            pt[aa][psl, lo:lo + 512 - skip],
                            start=(aa == 0), stop=(aa == last),
                            tile_position=tp,
                            skip_group_check=True,
                        )
                o_sb = outs.tile([65, 512], fp32, tag="o_sb")
                nc.vector.tensor_copy(o_sb, po_x[0:65, :])
                nc.vector.tensor_add(o_sb, o_sb, po_y[0:65, :])
                nc.sync.dma_start(
                    out=o_ap[:, 512 * j:512 * (j + 1)], in_=o_sb)

            # ---------- emission schedule ----------
            quarter_psum = {}

            def open_quarter(q):
                pkq = ps_kq.tile([128, QW], fp32, tag="kq")
                pv = ps_v.tile([128, 8, 64], fp32, tag="v")
                quarter_psum[q] = (pkq, pv)

            # quarters 0,1: pure load+project
            xqs = {}
            xqs[0] = emit_quarter_dma(0)
            xqs[1] = emit_quarter_dma(1)
            open_quarter(0)
            for k in range(KCH):
                emit_quarter_mm(0, xqs[0], k)
            emit_quarter_evac(0)
            open_quarter(1)
            for k in range(KCH):
                emit_quarter_mm(1, xqs[1], k)
            emit_quarter_evac(1)

            # quarters 2,3 interleaved with stage-1 score pairs (which only
            # need kq quarters 0-1); pairs fill PE gaps while DMA streams.
            xqs[2] = emit_quarter_dma(2)
            open_quarter(2)
            for k in range(KCH):
                emit_quarter_mm(2, xqs[2], k)
                if k in (2, 5):
                    emit_pair_scores(ps_s1, {2: 0, 5: 1}[k], stage=1, gmax=2)
            emit_quarter_evac(2)
            xqs[3] = emit_quarter_dma(3)
            open_quarter(3)
            for k in range(KCH):
                emit_quarter_mm(3, xqs[3], k)
                if k in (1, 3, 5, 7):
                    emit_pair_scores(ps_s1, {1: 2, 3: 3, 5: 4, 7: 5}[k], stage=1, gmax=2)
            emit_quarter_evac(3)
            emit_pair_scores(ps_s1, 6, stage=1, gmax=2)
            emit_pair_scores(ps_s1, 7, stage=1, gmax=2)

            s1stack.close()
            phase1.close()

            # stage 2: remaining scores + PV, all PSUM now free
            phase2 = ExitStack()
            ps_s2 = phase2.enter_context(
                tc.tile_pool(name="ps_s2", bufs=2, space="PSUM"))
            ps_o = phase2.enter_context(
                tc.tile_pool(name="ps_o", bufs=1, space="PSUM"))
            for p in range(16):
                emit_pair_scores(ps_s2, p, stage=2, gmax=3)
                # O^T t-tile j needs chunks <= 4j+3 (pairs <= 2j+1); lag one
                # pair so PE isn't head-of-line blocked on this pair's exps
                if p >= 2 and p % 2 == 0:
                    emit_pv((p - 2) // 2)
            emit_pv(7)
            phase2.close()
            ptp.close()

    nc.compile()
    return nc


def _get_nc():
    if "nc" not in _CACHE:
        _CACHE["nc"] = _build_program()
    return _CACHE["nc"]


def _in_maps(x, W):
    x = np.asarray(x, dtype=np.float32)
    W = np.asarray(W, dtype=np.float32)
    assert x.shape == (B, T, EMB) and W.shape == (EMB, 3 * HEAD)

    xt = x.transpose(0, 2, 1)  # [B, EMB, T]
    # quarter-contiguous: [B, NQ, EMB, QW] so each DMA line is contiguous
    xtq = np.ascontiguousarray(
        xt.reshape(B, EMB, NQ, QW).transpose(0, 2, 1, 3)
    ).astype(BF16)
    w16 = W.astype(BF16)
    mask = np.triu(np.ones((128, 128), np.float32)).astype(BF16)
    return [{"xt": xtq[b], "w": w16, "mask": mask} for b in range(B)]


def kernel(x, W):
    from concourse.bass_utils import run_bass_kernel_spmd

    nc = _get_nc()
    res = run_bass_kernel_spmd(nc, _in_maps(x, W), list(range(B)))
    out = []
    for b in range(B):
        ot = np.asarray(res.results[b]["o"], dtype=np.float32)  # [65, T]
        out.append((ot[0:HEAD] / ot[HEAD:HEAD + 1]).T)  # [T, HEAD]
    return np.stack(out).astype(np.float32)
